# revision 11
# baseline (speedup 1.0000x reference)
"""Gaussian mixture loss on 8 Trainium2 NeuronCores (Bass/Tile).

Math: for each predicted point p and gt means g_m,
    ll(p) = logsumexp_m( -C - ||p - g_m||^2 / 2 ),   C = 0.5*log(2*pi)
    loss  = -mean(ll)
Since all exponents are <= -C, exp never overflows and underflow is
harmless, so no max-subtraction is needed.

Kernel strategy (per core):
  - core c handles batch b=c//2, rows (c%2)*2048..+2048, all 4096 gt means
  - scores u[n,m] = c'*(2 p.g - ||g||^2 - ||p||^2), c' = 0.5*log2(e), via
    K=21 fp16 matmuls (hi/lo split keeps ~2^-22 relative accuracy); psum
    holds u so that exp(0.5 t) = 2^u = exp(u * ln2).
  - lhsT/rhs prep: aug tensors built with a handful of wide DVE ops, then
    per-chunk transposes: PE transpose + DVE copy for the early chunks
    (low latency), DMA-xbar transposes (SBUF->SBUF, engine-free) for the
    rest.
  - exp: most half-tiles on ACT (scale=ln2, fused accum row-sum); a
    tunable subset is offloaded to a DVE fast-exp2 chain (one 1x
    psum->fp16 pass, then 4x fp16 ops: magic-round, fraction, 2^r via
    int16 bit-build, quadratic poly, fused multiply + accum row-sum),
    with one op optionally on GPSIMD to balance engines.
  - ln + row-sum fused in one ACT instruction; partition_all_reduce;
    scalar out.  Host: loss = C - (sum of 8 partial sums) / 16384.
"""

import numpy as np

import concourse.bacc as bacc_mod
import concourse.tile as tile
from concourse import bacc, hw_specs, mybir
from concourse.bass_isa import ReduceOp
from concourse.bass_utils import run_bass_kernel_spmd
from concourse.masks import make_identity


def _patched_activation_tables(module_arch):
    """Steer Bacc's act-table-load chooser to the one set that contains
    BOTH Exp and Ln ("natural_log_exp_and_others"), so the kernel pays a
    single ACT_TABLE_LOAD at t=0 instead of an exp load at start plus a
    ~1.3us ln load on the critical tail."""
    both = {mybir.ActivationFunctionType.Exp, mybir.ActivationFunctionType.Ln}
    out = {}
    for name, funcs in hw_specs.get_activation_tables(module_arch).items():
        if name != "natural_log_exp_and_others":
            funcs = set(funcs) - both
        out[name] = funcs
    return out

# Problem shape (hardcoded per contract)
B, N, M, D = 4, 4096, 4096, 3
NCORES = 8
CORES_PER_BATCH = NCORES // B          # 2
RPC = N // CORES_PER_BATCH             # 2048 rows per core
CONST = 0.5 * np.log(2.0 * np.pi)

P = 128                                # partitions
NP_CH = RPC // P                       # 16 p-chunks per core
NG_CH = M // P                         # 32 g-chunks per core
KAUG = 21                              # augmented contraction dim
AUGW = 128                             # padded aug cols (xbar transpose needs 128)
HALF = M // 2                          # 2048 cols per main ACT instruction

CPRIME = float(0.5 * np.log2(np.e))    # psum scale: u = c' * t
LN2 = float(np.log(2.0))

# fast-exp2 poly for 2^-f on [-0.5, 0.5]: 1 + C1*f + C2*f^2 (LSQ fit)
C1 = -0.7015094
C2 = 0.24194956
MAGIC = 1551.0                         # 1536 + 15: fp16 round-to-int magic

F32 = mybir.dt.float32
F16 = mybir.dt.float16
I16 = mybir.dt.int16

# chunks transposed via PE+copy (low latency, used early); rest via DMA xbar
PE_TR_G = 16                           # g-chunks 0..15 (needed by all h=0 groups)
PE_TR_P = 4                            # p-chunks 0..3

# half-tiles offloaded to the DVE fast-exp2 chain: set of (i, h)
DVE_HALVES = {(5, 0), (8, 0), (11, 0), (14, 0),
              (2, 1), (5, 1), (8, 1), (11, 1)}
# offload halves whose bit-build (m) op also runs on gpsimd
M_ON_POOL = set()


def _build_g(nc, eng, raw, aug, sc):
    """g-side aug rows for chunks [c0:c1): k 0-2 g_hi, 3-5 g_hi dup,
    6-8 g_lo, 9-11 (-c'g^2)_hi, 12-14 lo, 15-20 ones."""
    neg = sc.tile_like(raw, tag="bscr")
    eng.tensor_scalar(out=neg, in0=raw, scalar1=-CPRIME, scalar2=None,
                      op0=mybir.AluOpType.mult)                 # -c'g
    nsq = sc.tile_like(raw, tag="bscr2")
    eng.tensor_tensor(out=nsq, in0=neg, in1=raw,
                      op=mybir.AluOpType.mult)                  # -c'g^2
    eng.tensor_copy(out=aug[:, :, 0:3], in_=raw)                # g_hi
    eng.tensor_copy(out=aug[:, :, 3:6], in_=aug[:, :, 0:3])     # dup
    eng.tensor_sub(out=aug[:, :, 6:9], in0=raw, in1=aug[:, :, 0:3])
    eng.tensor_copy(out=aug[:, :, 9:12], in_=nsq)               # hi
    eng.tensor_sub(out=aug[:, :, 12:15], in0=nsq, in1=aug[:, :, 9:12])
    eng.memset(aug[:, :, 15:KAUG], 1.0)


def _build_p(nc, eng, raw, aug, sc):
    """p-side aug rows: k 0-2 (2c'p)_hi, 3-5 lo, 6-8 hi dup, 9-14 ones,
    15-17 (-c'p^2)_hi, 18-20 lo."""
    neg = sc.tile_like(raw, tag="bscr")
    eng.tensor_scalar(out=neg, in0=raw, scalar1=-CPRIME, scalar2=None,
                      op0=mybir.AluOpType.mult)                 # -c'p
    nsq = sc.tile_like(raw, tag="bscr2")
    eng.tensor_tensor(out=nsq, in0=neg, in1=raw,
                      op=mybir.AluOpType.mult)                  # -c'p^2
    cp = sc.tile_like(raw, tag="bscr3")
    eng.tensor_scalar(out=cp, in0=raw, scalar1=2.0 * CPRIME, scalar2=None,
                      op0=mybir.AluOpType.mult)                 # 2c'p
    eng.tensor_copy(out=aug[:, :, 0:3], in_=cp)                 # hi
    eng.tensor_sub(out=aug[:, :, 3:6], in0=cp, in1=aug[:, :, 0:3])
    eng.tensor_copy(out=aug[:, :, 6:9], in_=aug[:, :, 0:3])     # dup
    eng.memset(aug[:, :, 9:15], 1.0)
    eng.tensor_copy(out=aug[:, :, 15:18], in_=nsq)              # hi
    eng.tensor_sub(out=aug[:, :, 18:KAUG], in0=nsq,
                   in1=aug[:, :, 15:18])


def build_program():
    nc = bacc.Bacc(
        "TRN2",
        target_bir_lowering=False,
        debug=False,
        num_devices=NCORES,
    )
    pred_h = nc.dram_tensor("pred", [RPC, D], F32, kind="ExternalInput").ap()
    gt_h = nc.dram_tensor("gt", [M, D], F32, kind="ExternalInput").ap()
    # per-(partition, p-chunk) partial sums; host does ln + reductions
    out_h = nc.dram_tensor("out", [P, 2 * NP_CH + 3], F32,
                           kind="ExternalOutput").ap()

    pred_view = pred_h.rearrange("(c p) d -> p c d", p=P)   # [128, 16, 3]
    gt_view = gt_h.rearrange("(c p) d -> p c d", p=P)       # [128, 32, 3]

    with tile.TileContext(nc) as tc:
        with (
            tc.tile_pool(name="consts", bufs=1) as consts,
            tc.tile_pool(name="work", bufs=3) as work,
            tc.tile_pool(name="psum", bufs=2, space="PSUM") as psum,
        ):
            # Fast-path loads for the first g/p batch, plus bulk loads.
            raw_g0 = work.tile([P, PE_TR_G, D], F32, tag="rawg0")
            nc.sync.dma_start(out=raw_g0, in_=gt_view[:, 0:PE_TR_G, :])
            raw_p0 = work.tile([P, PE_TR_P, D], F32, tag="rawp0")
            nc.sync.dma_start(out=raw_p0, in_=pred_view[:, 0:PE_TR_P, :])
            g_all = consts.tile([P, NG_CH, D], F32, tag="gall")
            nc.sync.dma_start(out=g_all, in_=gt_view)
            p_all = consts.tile([P, NP_CH, D], F32, tag="pall")
            nc.sync.dma_start(out=p_all, in_=pred_view)

            identity = consts.tile([P, P], F16, tag="ident")
            make_identity(nc, identity)

            # Warm the ACT exp table at t=0.
            warm = consts.tile([P, 1], F32, tag="warm")
            nc.vector.memset(warm, 1.0)
            nc.scalar.activation(out=warm, in_=warm,
                                 func=mybir.ActivationFunctionType.Exp)

            # aug tensors: padded to 128 cols for the DMA xbar transpose.
            aug_g = consts.tile([P, NG_CH, AUGW], F16, tag="augg")
            aug_p = consts.tile([P, NP_CH, AUGW], F16, tag="augp")
            # pad cols (never read by matmuls, but keep them finite);
            # gpsimd is idle at t=0 and can't touch the build cols anyway.
            nc.gpsimd.memset(aug_g[:, :, KAUG:AUGW], 0.0)
            nc.gpsimd.memset(aug_p[:, :, KAUG:AUGW], 0.0)

            # transposed chunk tiles: rows 0:21 = k, cols = 128 points
            gT = [consts.tile([P, P], F16, tag=f"gT{j}", name=f"gT{j}")
                  for j in range(NG_CH)]
            pT = [consts.tile([P, P], F16, tag=f"pT{i}", name=f"pT{i}")
                  for i in range(NP_CH)]

            # acc: col h*16+i per main half-tile + 3 opening scratch cols
            NACC = 2 * NP_CH
            acc = consts.tile([P, NACC + 3], F32, tag="acc")

            # ---- builds ----
            # fast path on DVE: early chunks from the small quick loads
            _build_g(nc, nc.vector, raw_g0, aug_g[:, 0:PE_TR_G, :], work)
            _build_p(nc, nc.vector, raw_p0, aug_p[:, 0:PE_TR_P, :], work)
            # bulk on gpsimd (keeps DVE free for the opening tr copies)
            _build_p(nc, nc.gpsimd, p_all[:, PE_TR_P:, :],
                     aug_p[:, PE_TR_P:, :], work)
            _build_g(nc, nc.gpsimd, g_all[:, PE_TR_G:, :],
                     aug_g[:, PE_TR_G:, :], work)

            # ---- transposes ----
            def pe_tr(aug_slice, dst):
                ps = psum.tile([KAUG, P], F16, tag="mm")
                nc.tensor.transpose(ps, aug_slice, identity)
                nc.vector.tensor_copy(out=dst[0:KAUG, :], in_=ps)

            def dma_tr(aug_slice, dst):
                nc.sync.dma_start_transpose(dst, aug_slice)

            JPH = HALF // P               # 16 g-chunks per half-tile

            def matmuls(pt, i, h, j0, j1):
                for j in range(j0, j1):
                    nc.tensor.matmul(
                        pt[:, (j - j0) * P:(j - j0 + 1) * P],
                        lhsT=pT[i][0:KAUG, :],
                        rhs=gT[h * JPH + j][0:KAUG, :],
                        start=True, stop=True,
                    )

            def act_group(i, h):
                pt = psum.tile([P, HALF], F32, tag="mm")
                matmuls(pt, i, h, 0, JPH)
                col = h * NP_CH + i
                nc.scalar.activation(
                    out=pt, in_=pt, func=mybir.ActivationFunctionType.Exp,
                    bias=0.0, scale=LN2,
                    accum_out=acc[:, col:col + 1],
                )

            def dve_group(i, h):
                """fast-exp2 on DVE (+1 op on gpsimd): one 1x psum->fp16
                pass, then packed-fp16 4x ops with fused accum row-sum."""
                pt = psum.tile([P, HALF], F32, tag="mm")
                matmuls(pt, i, h, 0, JPH)
                u16 = work.tile([P, HALF], F16, tag="u16", bufs=2)
                nc.vector.tensor_copy(out=u16, in_=pt)              # 1x
                w = work.tile([P, HALF], F16, tag="w", bufs=2)
                nc.gpsimd.tensor_scalar(                            # pool
                    out=w, in0=u16, scalar1=-15.0, scalar2=MAGIC,
                    op0=mybir.AluOpType.max, op1=mybir.AluOpType.add)
                fp = work.tile([P, HALF], F16, tag="fp", bufs=2)
                nc.vector.scalar_tensor_tensor(                     # r - u
                    out=fp, in0=w, scalar=MAGIC, in1=u16,
                    op0=mybir.AluOpType.subtract,
                    op1=mybir.AluOpType.subtract)
                m = work.tile([P, HALF], I16, tag="m", bufs=2)
                meng = nc.gpsimd if (i, h) in M_ON_POOL else nc.vector
                meng.tensor_scalar(                                 # (r+15)*1024
                    out=m, in0=w, scalar1=-1536.0, scalar2=1024.0,
                    op0=mybir.AluOpType.add, op1=mybir.AluOpType.mult)
                p1 = work.tile([P, HALF], F16, tag="p1", bufs=2)
                nc.vector.tensor_scalar(                            # C2*f + C1
                    out=p1, in0=fp, scalar1=C2, scalar2=C1,
                    op0=mybir.AluOpType.mult, op1=mybir.AluOpType.add)
                q = work.tile([P, HALF], F16, tag="q", bufs=2)
                nc.vector.scalar_tensor_tensor(                     # p1*f
                    out=q, in0=p1, scalar=1.0, in1=fp,
                    op0=mybir.AluOpType.mult, op1=mybir.AluOpType.mult)
                exd = work.tile([P, HALF], F16, tag="exd", bufs=2)
                col = h * NP_CH + i
                nc.vector.scalar_tensor_tensor(                     # (q+1)*2^r
                    out=exd, in0=q, scalar=1.0, in1=m.bitcast(F16),
                    op0=mybir.AluOpType.add, op1=mybir.AluOpType.mult,
                    accum_out=acc[:, col:col + 1])

            # ---- opening: PE-path transposes interleaved with chunk 0
            # half 0 processed as 4 x 512-wide slices so the first exp
            # fires as soon as the first 4 g-chunks are transposed ----
            pe_tr(aug_p[:, 0, 0:KAUG], pT[0])
            for t in range(4):
                pe_tr(aug_g[:, t, 0:KAUG], gT[t])
            for qround in range(4):
                ptq = psum.tile([P, 4 * P], F32, tag="mm",
                                name=f"pt0_{qround}")
                matmuls(ptq, 0, 0, 4 * qround, 4 * qround + 4)
                col = [NACC, NACC + 1, NACC + 2, 0][qround]
                nc.scalar.activation(
                    out=ptq, in_=ptq,
                    func=mybir.ActivationFunctionType.Exp,
                    bias=0.0, scale=LN2,
                    accum_out=acc[:, col:col + 1],
                )
                if qround < 3:
                    for t in range(4 * qround + 4, 4 * qround + 8):
                        pe_tr(aug_g[:, t, 0:KAUG], gT[t])
                else:
                    for t in range(1, PE_TR_P):
                        pe_tr(aug_p[:, t, 0:KAUG], pT[t])
            # DMA xbar transposes for the rest: p first (needed sooner)
            for t in range(PE_TR_P, NP_CH):
                dma_tr(aug_p[:, t, :], pT[t])
            for t in range(PE_TR_G, NG_CH):
                dma_tr(aug_g[:, t, :], gT[t])

            # ---- main loop ----
            for i in range(1, NP_CH):
                if (i, 0) in DVE_HALVES:
                    dve_group(i, 0)
                else:
                    act_group(i, 0)
            for i in range(NP_CH):
                if (i, 1) in DVE_HALVES:
                    dve_group(i, 1)
                else:
                    act_group(i, 1)

            # ---- finalize: ship the partial sums; ln + reductions on host
            nc.sync.dma_start(out=out_h, in_=acc)

    orig_tables = bacc_mod.get_activation_tables
    bacc_mod.get_activation_tables = _patched_activation_tables
    try:
        nc.compile()
    finally:
        bacc_mod.get_activation_tables = orig_tables
    return nc


_NC_CACHE = {}


def run(predicted_points, gt_means, trace=False, **trace_kwargs):
    """Shard inputs, run the SPMD bass kernel, gather. Returns
    (loss_scalar_f32, BassKernelResults)."""
    pred = np.ascontiguousarray(np.asarray(predicted_points, dtype=np.float32))
    gt = np.ascontiguousarray(np.asarray(gt_means, dtype=np.float32))
    assert pred.shape == (B, N, D) and gt.shape == (B, M, D)

    if "nc" not in _NC_CACHE:
        _NC_CACHE["nc"] = build_program()
    nc = _NC_CACHE["nc"]

    in_maps = []
    for c in range(NCORES):
        b = c // CORES_PER_BATCH
        r0 = (c % CORES_PER_BATCH) * RPC
        in_maps.append({
            "pred": np.ascontiguousarray(pred[b, r0:r0 + RPC, :]),
            "gt": np.ascontiguousarray(gt[b]),
        })

    res = run_bass_kernel_spmd(nc, in_maps, list(range(NCORES)),
                               trace=trace, **trace_kwargs)
    total = 0.0
    for c in range(NCORES):
        acc = np.asarray(res.results[c]["out"], dtype=np.float64)
        S = acc[:, 0:NP_CH] + acc[:, NP_CH:2 * NP_CH]
        S[:, 0] += acc[:, 2 * NP_CH:].sum(axis=1)
        total += np.log(S).sum()
    loss = np.asarray(CONST - total / (B * N), dtype=np.float32)
    return loss, res


def kernel(predicted_points, gt_means):
    loss, _ = run(predicted_points, gt_means, trace=False)
    return loss


# revision 19
# speedup vs baseline: 1.5971x; 1.5971x over previous
"""Gaussian mixture loss on 8 Trainium2 NeuronCores (Bass/Tile).

Math: for each predicted point p and gt means g_m,
    ll(p) = logsumexp_m( -C - ||p - g_m||^2 / 2 ),   C = 0.5*log(2*pi)
    loss  = -mean(ll)
Since all exponents are <= -C, exp never overflows and underflow is
harmless, so no max-subtraction is needed.

Kernel strategy (per core):
  - core c handles batch b=c//2, rows (c%2)*2048..+2048, all 4096 gt means
  - scores u[n,m] = c'*(2 p.g - ||g||^2 - ||p||^2), c' = 0.5*log2(e), via
    K=21 fp16 matmuls (hi/lo split keeps ~2^-22 relative accuracy); psum
    holds u so that exp(0.5 t) = 2^u = exp(u * ln2).
  - lhsT/rhs prep: aug tensors built with a handful of wide DVE ops, then
    per-chunk transposes: PE transpose + DVE copy for the early chunks
    (low latency), DMA-xbar transposes (SBUF->SBUF, engine-free) for the
    rest.
  - exp: most half-tiles on ACT (scale=ln2, fused accum row-sum); a
    tunable subset is offloaded to a DVE fast-exp2 chain (one 1x
    psum->fp16 pass, then 4x fp16 ops: magic-round, fraction, 2^r via
    int16 bit-build, quadratic poly, fused multiply + accum row-sum),
    with one op optionally on GPSIMD to balance engines.
  - ln + row-sum fused in one ACT instruction; partition_all_reduce;
    scalar out.  Host: loss = C - (sum of 8 partial sums) / 16384.
"""

import numpy as np

import concourse.bacc as bacc_mod
import concourse.tile as tile
from concourse import bacc, hw_specs, mybir
from concourse.bass_isa import ReduceOp
from concourse.bass_utils import run_bass_kernel_spmd
from concourse.masks import make_identity


def _patched_activation_tables(module_arch):
    """Steer Bacc's act-table-load chooser to the one set that contains
    BOTH Exp and Ln ("natural_log_exp_and_others"), so the kernel pays a
    single ACT_TABLE_LOAD at t=0 instead of an exp load at start plus a
    ~1.3us ln load on the critical tail."""
    both = {mybir.ActivationFunctionType.Exp, mybir.ActivationFunctionType.Ln}
    out = {}
    for name, funcs in hw_specs.get_activation_tables(module_arch).items():
        if name != "natural_log_exp_and_others":
            funcs = set(funcs) - both
        out[name] = funcs
    return out

# Problem shape (hardcoded per contract)
B, N, M, D = 4, 4096, 4096, 3
NCORES = 8
CORES_PER_BATCH = NCORES // B          # 2
RPC = N // CORES_PER_BATCH             # 2048 rows per core
CONST = 0.5 * np.log(2.0 * np.pi)

P = 128                                # partitions
NP_CH = RPC // P                       # 16 p-chunks per core
NG_CH = M // P                         # 32 g-chunks per core
KAUG = 22                              # augmented contraction dim (incl magic)
AUGW = 128                             # padded aug cols (xbar transpose needs 128)
HALF = M // 2                          # 2048 cols per main ACT instruction

# psum holds u' = 1024*(0.5*log2(e)*t + 15) = 1024*log2(sqrt(e^t)) + 15360;
# exp(0.5 t) = 2^((u'-15360)/1024).  The +15360 comes from the magic K-row
# (120 * 128); the DVE fast path bit-casts int16(u') as fp16 => 2^r*(1+f).
CPP = float(1024.0 * 0.5 * np.log2(np.e))
ASCALE = float(np.log(2.0) / 1024.0)
ABIAS = float(-15360.0 * np.log(2.0) / 1024.0)
MROW_P = 120.0                         # p-side magic aug value
MROW_G = 128.0                         # g-side magic aug value
# host-side mean correction for the linear-interp fast exp: E[(1+f)/2^f]
KCORR_ROUND = 0.960581
KCORR_TRUNC = 0.960906

F32 = mybir.dt.float32
F16 = mybir.dt.float16
I16 = mybir.dt.int16

# chunks transposed via PE+copy (low latency, used early); rest via DMA xbar
PE_TR_G = 16                           # g-chunks 0..15 (needed by all h=0 groups)
PE_TR_P = 4                            # p-chunks 0..3

# half-tiles offloaded to the DVE fast-exp2 chain: set of (i, h)
DVE_HALVES = {(4, 0), (7, 0), (10, 0), (13, 0), (15, 0),
              (1, 1), (4, 1), (7, 1), (10, 1), (13, 1), (15, 1)}
# offload halves whose bitcast row-sum op runs on gpsimd instead of DVE
B_ON_POOL = set()


def _build_g(nc, eng, raw, aug, sc):
    """g-side aug rows: k 0-2 g_hi, 3-5 g_hi dup, 6-8 g_lo,
    9-11 (-c''g^2)_hi, 12-14 lo, 15-20 ones, 21 magic (128)."""
    neg = sc.tile_like(raw, tag="bscr")
    eng.tensor_scalar(out=neg, in0=raw, scalar1=-CPP, scalar2=None,
                      op0=mybir.AluOpType.mult)                 # -c''g
    nsq = sc.tile_like(raw, tag="bscr2")
    eng.tensor_tensor(out=nsq, in0=neg, in1=raw,
                      op=mybir.AluOpType.mult)                  # -c''g^2
    eng.tensor_copy(out=aug[:, :, 0:3], in_=raw)                # g_hi
    eng.tensor_copy(out=aug[:, :, 3:6], in_=aug[:, :, 0:3])     # dup
    eng.tensor_sub(out=aug[:, :, 6:9], in0=raw, in1=aug[:, :, 0:3])
    eng.tensor_copy(out=aug[:, :, 9:12], in_=nsq)               # hi
    eng.tensor_sub(out=aug[:, :, 12:15], in0=nsq, in1=aug[:, :, 9:12])
    eng.memset(aug[:, :, 15:21], 1.0)
    eng.memset(aug[:, :, 21:KAUG], MROW_G)


def _build_p(nc, eng, raw, aug, sc):
    """p-side aug rows: k 0-2 (2c''p)_hi, 3-5 lo, 6-8 hi dup, 9-14 ones,
    15-17 (-c''p^2)_hi, 18-20 lo, 21 magic (120)."""
    neg = sc.tile_like(raw, tag="bscr")
    eng.tensor_scalar(out=neg, in0=raw, scalar1=-CPP, scalar2=None,
                      op0=mybir.AluOpType.mult)                 # -c''p
    nsq = sc.tile_like(raw, tag="bscr2")
    eng.tensor_tensor(out=nsq, in0=neg, in1=raw,
                      op=mybir.AluOpType.mult)                  # -c''p^2
    cp = sc.tile_like(raw, tag="bscr3")
    eng.tensor_scalar(out=cp, in0=raw, scalar1=2.0 * CPP, scalar2=None,
                      op0=mybir.AluOpType.mult)                 # 2c''p
    eng.tensor_copy(out=aug[:, :, 0:3], in_=cp)                 # hi
    eng.tensor_sub(out=aug[:, :, 3:6], in0=cp, in1=aug[:, :, 0:3])
    eng.tensor_copy(out=aug[:, :, 6:9], in_=aug[:, :, 0:3])     # dup
    eng.memset(aug[:, :, 9:15], 1.0)
    eng.tensor_copy(out=aug[:, :, 15:18], in_=nsq)              # hi
    eng.tensor_sub(out=aug[:, :, 18:21], in0=nsq,
                   in1=aug[:, :, 15:18])
    eng.memset(aug[:, :, 21:KAUG], MROW_P)


def build_program():
    nc = bacc.Bacc(
        "TRN2",
        target_bir_lowering=False,
        debug=False,
        num_devices=NCORES,
    )
    pred_h = nc.dram_tensor("pred", [RPC, D], F32, kind="ExternalInput").ap()
    gt_h = nc.dram_tensor("gt", [M, D], F32, kind="ExternalInput").ap()
    # per-(partition, p-chunk) partial sums; host does ln + reductions
    out_h = nc.dram_tensor("out", [P, 2 * NP_CH + 3], F32,
                           kind="ExternalOutput").ap()

    pred_view = pred_h.rearrange("(c p) d -> p c d", p=P)   # [128, 16, 3]
    gt_view = gt_h.rearrange("(c p) d -> p c d", p=P)       # [128, 32, 3]

    with tile.TileContext(nc) as tc:
        with (
            tc.tile_pool(name="consts", bufs=1) as consts,
            tc.tile_pool(name="work", bufs=3) as work,
            tc.tile_pool(name="psum", bufs=2, space="PSUM") as psum,
        ):
            # Fast-path loads for the first g/p batch, plus bulk loads.
            raw_g0 = work.tile([P, PE_TR_G, D], F32, tag="rawg0")
            nc.sync.dma_start(out=raw_g0, in_=gt_view[:, 0:PE_TR_G, :])
            raw_p0 = work.tile([P, PE_TR_P, D], F32, tag="rawp0")
            nc.sync.dma_start(out=raw_p0, in_=pred_view[:, 0:PE_TR_P, :])
            g_all = consts.tile([P, NG_CH, D], F32, tag="gall")
            nc.sync.dma_start(out=g_all, in_=gt_view)
            p_all = consts.tile([P, NP_CH, D], F32, tag="pall")
            nc.sync.dma_start(out=p_all, in_=pred_view)

            identity = consts.tile([P, P], F16, tag="ident")
            make_identity(nc, identity)

            # Warm the ACT exp table at t=0.
            warm = consts.tile([P, 1], F32, tag="warm")
            nc.vector.memset(warm, 1.0)
            nc.scalar.activation(out=warm, in_=warm,
                                 func=mybir.ActivationFunctionType.Exp)
            abias = consts.tile([P, 1], F32, tag="abias")
            nc.vector.memset(abias, ABIAS)
            ascale = consts.tile([P, 1], F32, tag="ascale")
            nc.vector.memset(ascale, ASCALE)

            # aug tensors: padded to 128 cols for the DMA xbar transpose.
            aug_g = consts.tile([P, NG_CH, AUGW], F16, tag="augg")
            aug_p = consts.tile([P, NP_CH, AUGW], F16, tag="augp")
            # pad cols (never read by matmuls, but keep them finite);
            # gpsimd is idle at t=0 and can't touch the build cols anyway.
            nc.gpsimd.memset(aug_g[:, :, KAUG:AUGW], 0.0)
            nc.gpsimd.memset(aug_p[:, :, KAUG:AUGW], 0.0)

            # transposed chunk tiles: rows 0:21 = k, cols = 128 points
            gT = [consts.tile([P, P], F16, tag=f"gT{j}", name=f"gT{j}")
                  for j in range(NG_CH)]
            pT = [consts.tile([P, P], F16, tag=f"pT{i}", name=f"pT{i}")
                  for i in range(NP_CH)]

            # acc: col h*16+i per main half-tile + 3 opening scratch cols
            NACC = 2 * NP_CH
            acc = consts.tile([P, NACC + 3], F32, tag="acc")

            # ---- builds ----
            # fast path on DVE: early chunks from the small quick loads
            _build_g(nc, nc.vector, raw_g0, aug_g[:, 0:PE_TR_G, :], work)
            _build_p(nc, nc.vector, raw_p0, aug_p[:, 0:PE_TR_P, :], work)
            # bulk on gpsimd (keeps DVE free for the opening tr copies)
            _build_p(nc, nc.gpsimd, p_all[:, PE_TR_P:, :],
                     aug_p[:, PE_TR_P:, :], work)
            _build_g(nc, nc.gpsimd, g_all[:, PE_TR_G:, :],
                     aug_g[:, PE_TR_G:, :], work)

            # ---- transposes ----
            def pe_tr(aug_slice, dst):
                ps = psum.tile([KAUG, P], F16, tag="mm")
                nc.tensor.transpose(ps, aug_slice, identity)
                nc.vector.tensor_copy(out=dst[0:KAUG, :], in_=ps)

            def dma_tr(aug_slice, dst):
                nc.sync.dma_start_transpose(dst, aug_slice)

            JPH = HALF // P               # 16 g-chunks per half-tile

            def matmuls(pt, i, h, j0, j1):
                for j in range(j0, j1):
                    nc.tensor.matmul(
                        pt[:, (j - j0) * P:(j - j0 + 1) * P],
                        lhsT=pT[i][0:KAUG, :],
                        rhs=gT[h * JPH + j][0:KAUG, :],
                        start=True, stop=True,
                    )

            def act_group(i, h):
                pt = psum.tile([P, HALF], F32, tag="mm")
                matmuls(pt, i, h, 0, JPH)
                col = h * NP_CH + i
                nc.scalar.activation(
                    out=pt, in_=pt, func=mybir.ActivationFunctionType.Exp,
                    bias=abias, scale=ascale,
                    accum_out=acc[:, col:col + 1],
                )

            def dve_group(i, h):
                """linear-interp fast exp2 on DVE: int16(max(u',0))
                bit-cast as fp16 is 2^r*(1+f); host multiplies the col by
                the mean correction KCORR."""
                pt = psum.tile([P, HALF], F32, tag="mm")
                matmuls(pt, i, h, 0, JPH)
                n16 = work.tile([P, HALF], I16, tag="n16", bufs=2)
                nc.vector.tensor_scalar(                            # 1x psum
                    out=n16, in0=pt, scalar1=0.0, scalar2=None,
                    op0=mybir.AluOpType.max)
                exd = work.tile([P, HALF], F16, tag="exd", bufs=2)
                col = h * NP_CH + i
                beng = nc.gpsimd if (i, h) in B_ON_POOL else nc.vector
                beng.tensor_scalar(                                 # 4x + accum
                    out=exd, in0=n16.bitcast(F16), scalar1=1.0,
                    scalar2=0.0, op0=mybir.AluOpType.mult,
                    op1=mybir.AluOpType.add,
                    accum_out=acc[:, col:col + 1])

            # ---- opening: PE-path transposes interleaved with chunk 0
            # half 0 processed as 4 x 512-wide slices so the first exp
            # fires as soon as the first 4 g-chunks are transposed ----
            pe_tr(aug_p[:, 0, 0:KAUG], pT[0])
            for t in range(4):
                pe_tr(aug_g[:, t, 0:KAUG], gT[t])
            for qround in range(4):
                ptq = psum.tile([P, 4 * P], F32, tag="mm",
                                name=f"pt0_{qround}")
                matmuls(ptq, 0, 0, 4 * qround, 4 * qround + 4)
                col = [NACC, NACC + 1, NACC + 2, 0][qround]
                nc.scalar.activation(
                    out=ptq, in_=ptq,
                    func=mybir.ActivationFunctionType.Exp,
                    bias=abias, scale=ascale,
                    accum_out=acc[:, col:col + 1],
                )
                if qround < 3:
                    for t in range(4 * qround + 4, 4 * qround + 8):
                        pe_tr(aug_g[:, t, 0:KAUG], gT[t])
                else:
                    for t in range(1, PE_TR_P):
                        pe_tr(aug_p[:, t, 0:KAUG], pT[t])
            # DMA xbar transposes for the rest: p first (needed sooner)
            for t in range(PE_TR_P, NP_CH):
                dma_tr(aug_p[:, t, :], pT[t])
            for t in range(PE_TR_G, NG_CH):
                dma_tr(aug_g[:, t, :], gT[t])

            # ---- main loop ----
            for i in range(1, NP_CH):
                if (i, 0) in DVE_HALVES:
                    dve_group(i, 0)
                else:
                    act_group(i, 0)
            for i in range(NP_CH):
                if (i, 1) in DVE_HALVES:
                    dve_group(i, 1)
                else:
                    act_group(i, 1)

            # ---- finalize: ship the partial sums; ln + reductions on host
            nc.sync.dma_start(out=out_h, in_=acc)

    orig_tables = bacc_mod.get_activation_tables
    bacc_mod.get_activation_tables = _patched_activation_tables
    try:
        nc.compile()
    finally:
        bacc_mod.get_activation_tables = orig_tables
    return nc


_NC_CACHE = {}


def run(predicted_points, gt_means, trace=False, **trace_kwargs):
    """Shard inputs, run the SPMD bass kernel, gather. Returns
    (loss_scalar_f32, BassKernelResults)."""
    pred = np.ascontiguousarray(np.asarray(predicted_points, dtype=np.float32))
    gt = np.ascontiguousarray(np.asarray(gt_means, dtype=np.float32))
    assert pred.shape == (B, N, D) and gt.shape == (B, M, D)

    if "nc" not in _NC_CACHE:
        _NC_CACHE["nc"] = build_program()
    nc = _NC_CACHE["nc"]

    in_maps = []
    for c in range(NCORES):
        b = c // CORES_PER_BATCH
        r0 = (c % CORES_PER_BATCH) * RPC
        in_maps.append({
            "pred": np.ascontiguousarray(pred[b, r0:r0 + RPC, :]),
            "gt": np.ascontiguousarray(gt[b]),
        })

    res = run_bass_kernel_spmd(nc, in_maps, list(range(NCORES)),
                               trace=trace, **trace_kwargs)
    # per-col fast-exp correction: DVE-offloaded cols need KCORR
    kcol = np.ones(2 * NP_CH + 3)
    for (i, h) in DVE_HALVES:
        kcol[h * NP_CH + i] = KCORR_TRUNC
    total = 0.0
    for c in range(NCORES):
        acc = np.asarray(res.results[c]["out"], dtype=np.float64) * kcol
        S = acc[:, 0:NP_CH] + acc[:, NP_CH:2 * NP_CH]
        S[:, 0] += acc[:, 2 * NP_CH:].sum(axis=1)
        total += np.log(S).sum()
    loss = np.asarray(CONST - total / (B * N), dtype=np.float32)
    return loss, res


def kernel(predicted_points, gt_means):
    loss, _ = run(predicted_points, gt_means, trace=False)
    return loss
